# revision 17
# baseline (speedup 1.0000x reference)
"""Deep-hedging GRU recurrence kernel for 8 Trainium2 NeuronCores.

Strategy: pure data-parallel over n_sim paths (8192 paths/core). The
per-step GRU folds the scalar hedge position d_t into the recurrence
weights (d_t = W_out @ h_t + b_out substitutes into the gate input),
leaving a pure 64-dim GRU in h with modified weights.

Software-pipelined (skewed) emission: the per-pair chain
  MM -> sigmoid/gn_copy -> u -> v -> tanh -> t1 -> t2 -> (+n accum)
zigzags across 4 engines; emitting it pair-by-pair serializes the whole
step on the in-order engine queues (22.6 ms). Instead each stage of
pair p is emitted in slot p+lag, so every engine's queue interleaves
pairs and dependencies point >=1 slot back (6.7 ms measured).

Everything is bf16 (R state, weights, elementwise); matmuls accumulate
fp32 in PSUM. Even pairs use R layout [x(0:4); 1(4); 0; h(64:128)];
odd pairs the mirrored layout [h(0:64); x(64:68); 1(68); 0] with
row/col-permuted weights. The weight M-permutations put r and h_n in
the block opposite h, and z and i_n in h's block, so every vector TT
has equal input base partitions (both-SBUF requirement) and runs in
the 2x bf16 mode; outputs rebase for free. The v-tensors of an
(even, odd) couple pack into one [128, 1024] tile -> one tanh per two
pairs. h' is finished by a gpsimd SWDGE dma-accumulate
(R_nxt = z*(h-n), then R_nxt += n) to keep the op off the DVE.

d_t (output D[t-1]) is computed from R_cur (which holds h_t) in the
same slot as the gate matmuls; 4 pairs x 2 chunks pack into one PSUM
bank at partitions 16j via tile_position col-groups plus a [128,17]
lhsT whose col-16 copy of w_d lands the second chunk at 32j+16. One
[128,512] copy + one partition-strided DMA drains 4 pairs' outputs.

Engine balance per slot (~3.3 us): tensor 4 gate MM + 2 d MM (N=512),
scalar sigmoid + g_n PSUM->SBUF bf16 copy + tanh/2, vector u/v/t1/t2
+ d-copy/4, gpsimd one dma-accum issue.
"""
import os
os.environ.setdefault("NEURON_RT_RESET_CORES", "1")
import sys
if "/opt/trn_rl_repo" not in sys.path:
    sys.path.insert(0, "/opt/trn_rl_repo")
import numpy as np

N_CORES = 8
N_SIM, N_STEP, IN_DIM, HID = 65536, 250, 4, 64
B_CORE = N_SIM // N_CORES      # 8192
CHUNK = 512                    # matmul free-dim chunk (one PSUM bank)
PAIR = 2 * CHUNK               # 1024
N_PAIR = B_CORE // PAIR        # 8


def _patch_tile_drain():
    """This walrus build rejects >1 sem-wait on a Drain TPB_CTRL; Bacc's
    generate_event_semaphores legalizes normal instructions but the Tile
    tail drain is emitted with the full global-clock wait set. Split those
    waits into standalone wait_ge instructions."""
    import concourse.tile as tile
    from concourse.vector_clock import ScopedClock

    if getattr(tile.TileContext, "_drain_patched", False):
        return

    def patched(self, tick_clock, wait_clock):
        nc = self.nc
        drain_inst = nc.sync.drain()
        wait_clock.add_sem_waits(
            drain_inst.ins, ScopedClock({None: tick_clock.global_clock})
        )
        inst = drain_inst.ins
        si = inst.sync_info
        waits = list(si.on_wait) if si and si.on_wait else []
        if si is not None:
            si.on_wait = []
        name2h = {h.name: h for h in self.sems.allocated().values()}
        for w in waits:
            assert w.wait_mode == "sem-ge-imm", w
            nc.sync.wait_ge(name2h[w.ant_name], w.wait_value)
        nc.all_engine_barrier()
        popped = nc._tile_sem_poison_stack.pop()
        assert popped is self._sem_poison
        nc.clear_and_free_semaphores(list(self.sems.allocated().values()))
        nc.all_engine_barrier()

    tile.TileContext._drain_and_barrier = patched
    tile.TileContext._drain_patched = True


def build_nc(b=B_CORE, n_step=N_STEP):
    """Build the per-core Bass program. b paths, n_step time steps."""
    import concourse.bacc as bacc
    import concourse.mybir as mybir
    import concourse.tile as tile
    from concourse.alu_op_type import AluOpType

    _patch_tile_drain()
    f32 = mybir.dt.float32
    bf16 = mybir.dt.bfloat16
    f32r = mybir.dt.float32r
    Act = mybir.ActivationFunctionType

    n_pair = b // PAIR
    nc = bacc.Bacc("TRN2", target_bir_lowering=False)

    # X pre-arranged host-side as [n_step, n_pair, IN_DIM, PAIR]
    X = nc.dram_tensor("X", [n_step, n_pair, IN_DIM, PAIR], bf16,
                       kind="ExternalInput")
    LW = nc.dram_tensor("LW", [12, 128, 128], bf16, kind="ExternalInput")
    INIT = nc.dram_tensor("INIT", [2, 128, PAIR], bf16, kind="ExternalInput")
    D = nc.dram_tensor("D", [n_step, b], f32, kind="ExternalOutput")
    D_q = D.ap().rearrange("t (q f) -> t q f", f=4 * PAIR)

    # weight index map: [rz_e, rz0_e, n_e, n0_e, d_e, rz_o, rz0_o, n_o, n0_o, d_o]
    W_RZ, W_RZ0, W_N, W_N0, W_D = 0, 1, 2, 3, 4

    with tile.TileContext(nc) as tc:
        with (
            tc.tile_pool(name="wp", bufs=1) as wp,
            tc.tile_pool(name="state", bufs=1) as state,
            tc.tile_pool(name="rzp", bufs=7) as rzp,
            tc.tile_pool(name="gnp", bufs=4) as gnp,
            tc.tile_pool(name="up", bufs=3) as up,
            tc.tile_pool(name="vp", bufs=4) as vp,
            tc.tile_pool(name="np_", bufs=5) as np_,
            tc.tile_pool(name="t1p", bufs=3) as t1p,
            tc.tile_pool(name="t2p", bufs=3) as t2p,
            tc.tile_pool(name="dsb", bufs=3) as dsb,
            tc.tile_pool(name="prz", bufs=2, space="PSUM") as prz,
            tc.tile_pool(name="pn", bufs=1, space="PSUM") as pn,
            tc.tile_pool(name="pd", bufs=2, space="PSUM") as pd,
        ):
            w = []
            for i in range(12):
                wt = wp.tile([128, 128], bf16, tag=f"w{i}", name=f"w{i}")
                nc.sync.dma_start(out=wt[:], in_=LW[i])
                w.append(wt)

            # per-pair, per-parity recurrent state tiles
            R = [[state.tile([128, PAIR], bf16, tag=f"R{par}_{p}", name=f"R{par}_{p}")
                  for p in range(n_pair)] for par in range(2)]
            for p in range(n_pair):
                lay = p % 2  # 0 = even layout, 1 = mirrored
                nc.sync.dma_start(out=R[0][p][:], in_=INIT[lay])
                nc.sync.dma_start(out=R[1][p][:], in_=INIT[lay])
                xs = slice(0, 4) if lay == 0 else slice(64, 68)
                nc.sync.dma_start(out=R[0][p][xs, :], in_=X[0, p])

            # per-slot in-flight structures, indexed by global slot k
            g_rz = {}
            g_n = {}
            d_ps = {}
            rz = {}
            gn = {}
            u_t = {}
            v_t = {}
            n_t = {}
            t1_t = {}
            t2_t = {}
            d_sb = {}

            def hs(lay):
                # h rows in R / elementwise partition block for this layout
                return slice(64, 128) if lay == 0 else slice(0, 64)

            def os_(lay):
                # "other" block (where i_n lives in gn, r lives in rz)
                return slice(0, 64) if lay == 0 else slice(64, 128)

            n_slot = n_step * n_pair
            for k in range(n_slot + n_pair + 8):
                t, p = divmod(k, n_pair)
                lay = p % 2
                in_main = k < n_slot

                # ---- lag 0: X prefetch + gate MMs + d MM (reads R_cur[p]) ----
                if in_main:
                    R_cur = R[t % 2][p]
                    R_nxt = R[(t + 1) % 2][p]
                    if t + 1 < n_step:
                        xs = slice(0, 4) if lay == 0 else slice(64, 68)
                        nc.sync.dma_start(out=R_nxt[xs, :], in_=X[t + 1, p])

                    wrz = w[5 * lay + (W_RZ0 if t == 0 else W_RZ)]
                    wn = w[5 * lay + (W_N0 if t == 0 else W_N)]
                    grz = prz.tile([128, PAIR], f32, tag="gd", name="grz")
                    nc.tensor.matmul(grz[:, 0:CHUNK], wrz[:], R_cur[:, 0:CHUNK],
                                     start=True, stop=True)
                    nc.tensor.matmul(grz[:, CHUNK:PAIR], wrz[:],
                                     R_cur[:, CHUNK:PAIR], start=True, stop=True)
                    g_rz[k] = grz
                if (in_main and t > 0) or (n_slot <= k < n_slot + n_pair):
                    # d_t-1; 4 consecutive pairs (a quad) pack into one PSUM
                    # tile at partitions 0/32/64/96 via tile_position.
                    tt, pp = divmod(k, n_pair)
                    j = pp % 4
                    R_d = R[tt % 2][pp]
                    wd17 = w[10 + (pp % 2)]
                    if j == 0:
                        dp = pd.tile([128, CHUNK], f32, tag="d", name="dp")
                        d_ps[k] = dp
                    else:
                        dp = d_ps[k - j]
                    # chunk c1 first: M=17 write puts d at partition 32j+16
                    # (cols 0:16 garbage); then c0's M=1 overwrites 32j.
                    nc.tensor.matmul(dp[32 * j:32 * j + 17, :], wd17[:, 0:17],
                                     R_d[:, CHUNK:PAIR], start=True, stop=True,
                                     tile_position=(0, 32 * j))
                    nc.tensor.matmul(dp[32 * j:32 * j + 1, :], wd17[:, 0:1],
                                     R_d[:, 0:CHUNK], start=True, stop=True,
                                     tile_position=(0, 32 * j))
                if in_main:
                    wn = w[5 * lay + (W_N0 if t == 0 else W_N)]
                    gn_ps = pn.tile([128, PAIR], f32, tag="gn", name="gn_ps")
                    nc.tensor.matmul(gn_ps[:, 0:CHUNK], wn[:], R_cur[:, 0:CHUNK],
                                     start=True, stop=True)
                    nc.tensor.matmul(gn_ps[:, CHUNK:PAIR], wn[:],
                                     R_cur[:, CHUNK:PAIR], start=True, stop=True)
                    g_n[k] = gn_ps

                # ---- lag 1: sigmoid + g_n copy (scalar), d copy ----
                c = k - 1
                if c >= 0 and c in g_rz:
                    rzt = rzp.tile([128, PAIR], bf16, tag="rz", name="rzt")
                    nc.scalar.activation(rzt[:], g_rz.pop(c)[:], Act.Sigmoid)
                    rz[c] = rzt
                    gnt = gnp.tile([128, PAIR], bf16, tag="gn", name="gnt")
                    nc.scalar.activation(gnt[:], g_n.pop(c)[:], Act.Copy)
                    gn[c] = gnt
                if c >= 0 and c % 4 == 3 and (c - 3) in d_ps:
                    dst = dsb.tile([128, CHUNK], f32, tag="d", name="dst")
                    nc.vector.tensor_copy(dst[:], d_ps.pop(c - 3)[:])
                    d_sb[c] = dst

                # ---- lag 2: u = r * h_n (vector) into the couple tile ----
                # r and h_n both live in the OTHER block (same base -> 2x
                # mode); the output rebases to the HOME block.
                c = k - 2
                if c >= 0 and c in rz:
                    tt, pp = divmod(c, n_pair)
                    ll = pp % 2
                    if ll == 0:
                        vt = vp.tile([128, PAIR], bf16, tag="v", name="vt")
                        v_t[c] = vt
                    else:
                        vt = v_t[c - 1]
                    nc.vector.tensor_tensor(vt[hs(ll), :], rz[c][os_(ll), :],
                                            gn[c][os_(ll), :], AluOpType.mult)
                    u_t[c] = vt
                if c in d_sb:
                    tt, pp = divmod(c, n_pair)
                    row = tt - 1 if tt < n_step else n_step - 1
                    q0 = pp - 3  # first pair of the quad
                    nc.sync.dma_start(
                        out=D_q[row, q0 // 4],
                        in_=d_sb.pop(c)[0:128:16, :])

                # ---- lag 3: v = u + i_n (vector, in place in couple tile) ----
                c = k - 3
                if c >= 0 and c in u_t:
                    tt, pp = divmod(c, n_pair)
                    ll = pp % 2
                    vt = u_t.pop(c)
                    nc.vector.tensor_tensor(vt[hs(ll), :], vt[hs(ll), :],
                                            gn[c][hs(ll), :], AluOpType.add)

                # ---- lag 4: tanh for the couple (odd member triggers) ----
                c = k - 4
                if c >= 0 and (c % n_pair) % 2 == 1 and (c - 1) in v_t:
                    nt = np_.tile([128, PAIR], bf16, tag="n", name="nt")
                    nc.scalar.activation(nt[:], v_t.pop(c - 1)[:], Act.Tanh)
                    n_t[c - 1] = n_t[c] = nt
                    # t1 for the even member (vector, uniform base)
                    tt, pp = divmod(c - 1, n_pair)
                    t1t = t1p.tile([128, PAIR], bf16, tag="t1", name="t1t")
                    nc.vector.tensor_tensor(t1t[64:128, :],
                                            R[tt % 2][pp][64:128, :],
                                            nt[64:128, :], AluOpType.subtract)
                    t1_t[c - 1] = t1t

                # ---- lag 5: t1 for the odd member ----
                c = k - 5
                if c >= 0 and (c % n_pair) % 2 == 1 and c in n_t:
                    tt, pp = divmod(c, n_pair)
                    t1o = t1p.tile([128, PAIR], bf16, tag="t1o", name="t1o")
                    nc.vector.tensor_tensor(t1o[0:64, :],
                                            R[tt % 2][pp][0:64, :],
                                            n_t[c][0:64, :], AluOpType.subtract)
                    t1_t[c] = t1o

                # ---- lag 6: R_nxt = t1 * z (vector, h' minus the n term) ----
                c = k - 6
                if c >= 0 and c in t1_t:
                    tt, pp = divmod(c, n_pair)
                    ll = pp % 2
                    R_nxt = R[(tt + 1) % 2][pp]
                    nc.vector.tensor_tensor(R_nxt[hs(ll), :],
                                            t1_t.pop(c)[hs(ll), :],
                                            rz[c][hs(ll), :], AluOpType.mult)
                    t2_t[c] = R_nxt
                    rz.pop(c)
                    gn.pop(c)

                # ---- lag 7: R_nxt += n via SWDGE dma accumulate ----
                c = k - 7
                if c >= 0 and c in t2_t:
                    tt, pp = divmod(c, n_pair)
                    ll = pp % 2
                    R_nxt = t2_t.pop(c)
                    nc.gpsimd.dma_start(out=R_nxt[hs(ll), :],
                                        in_=n_t[c][hs(ll), :],
                                        accum_op=AluOpType.add)
                    n_t.pop(c)
                    if ll == 1 and (c - 1) in n_t:
                        n_t.pop(c - 1)

    nc.finalize()
    return nc


def make_weights(W_in, b_in, W_ih, b_ih, W_hh, b_hh, W_out, b_out):
    """Fold d_t = W_out@h_t + b_out into the GRU weights; pack lhsT mats
    for the even layout [x(0:4); 1(4); 0(5:64); h(64:128)] and the
    mirrored odd layout [h(0:64); x(64:68); 1(68); 0(69:128)]."""
    A = W_ih[:, :64] @ W_in            # [192, 4]
    w_d = W_ih[:, 64]                  # [192]
    c_i = W_ih[:, :64] @ b_in + b_ih   # [192]
    Wh = W_hh + np.outer(w_d, W_out[0])
    Wo, bo = W_out[0], b_out[0]

    L_rz = np.zeros((128, 128), np.float32)
    L_rz[0:4] = A[:128].T
    L_rz[4] = c_i[:128] + b_hh[:128] + w_d[:128] * bo
    L_rz[64:128] = Wh[:128].T
    L_rz0 = L_rz.copy()
    L_rz0[4] = c_i[:128] + b_hh[:128]
    L_rz0[64:128] = W_hh[:128].T

    L_n = np.zeros((128, 128), np.float32)
    L_n[0:4, 0:64] = A[128:].T
    L_n[4, 0:64] = c_i[128:] + w_d[128:] * bo
    L_n[64:128, 0:64] = np.outer(Wo, w_d[128:])
    L_n[4, 64:128] = b_hh[128:]
    L_n[64:128, 64:128] = W_hh[128:].T
    L_n0 = L_n.copy()
    L_n0[4, 0:64] = c_i[128:]
    L_n0[64:128, 0:64] = 0

    L_d = np.zeros((128, 128), np.float32)
    L_d[4, 0] = bo
    L_d[64:128, 0] = Wo

    # Gate placement: for layout parity l with HOME = h-block of R and
    # OTHER = the opposite 64-partition block, the TensorTensor both-SBUF
    # same-base rule wants r and h_n in OTHER, and z and i_n in HOME.
    #   even (HOME=64:128): rz = [r; z] (identity), n = [h_n; i_n] (swap)
    #   odd  (HOME=0:64):   rz = [z; r] (swap),     n = [i_n; h_n] (identity)
    # Odd variants also permute K rows so h sits at 0:64, x at 64:68,
    # ones at 68.
    kperm = np.concatenate([np.arange(64, 128), np.arange(0, 4), [4],
                            np.arange(5, 64)]).astype(np.int64)
    mperm = np.concatenate([np.arange(64, 128), np.arange(0, 64)])
    # wd17 variants: w_d replicated at M-cols 0 and 16 so the two 512-col
    # chunks of a pair land at PSUM partitions 32j and 32j+16.
    wd17_e = np.zeros((128, 128), np.float32)
    wd17_e[:, 0] = L_d[:, 0]
    wd17_e[:, 16] = L_d[:, 0]
    wd17_o = np.zeros((128, 128), np.float32)
    wd17_o[:, 0] = L_d[kperm][:, 0]
    wd17_o[:, 16] = L_d[kperm][:, 0]
    outs = [L_rz, L_rz0, L_n[:, mperm], L_n0[:, mperm], L_d,
            L_rz[kperm][:, mperm], L_rz0[kperm][:, mperm],
            L_n[kperm], L_n0[kperm], L_d[kperm], wd17_e, wd17_o]
    return np.stack(outs)  # [12, 128, 128]


def make_init():
    init = np.zeros((2, 128, PAIR), np.float32)
    init[0, 4] = 1.0
    init[1, 68] = 1.0
    return init


_built = {}
_last_exec_ns = None


def kernel(X, W_in, b_in, W_ih, W_hh, b_ih, b_hh, W_out, b_out):
    from concourse.bass_utils import run_bass_kernel_spmd

    import ml_dtypes
    X = np.ascontiguousarray(np.asarray(X, dtype=np.float32))
    LW = make_weights(
        np.asarray(W_in), np.asarray(b_in), np.asarray(W_ih), np.asarray(b_ih),
        np.asarray(W_hh), np.asarray(b_hh), np.asarray(W_out), np.asarray(b_out))
    import ml_dtypes as _md
    initb = make_init().astype(_md.bfloat16)
    LWb = LW.astype(_md.bfloat16)

    key = (B_CORE, N_STEP)
    if key not in _built:
        _built[key] = build_nc(B_CORE, N_STEP)
    nc = _built[key]

    in_maps = []
    for c in range(N_CORES):
        Xc = X[c * B_CORE:(c + 1) * B_CORE]          # [B, T, 4]
        Xc = Xc.reshape(N_PAIR, PAIR, N_STEP, IN_DIM)
        Xc = np.ascontiguousarray(
            Xc.transpose(2, 0, 3, 1).astype(ml_dtypes.bfloat16))
        in_maps.append({"X": Xc, "LW": LWb, "INIT": initb})

    tmpdir = os.environ.get("BASS_TMPDIR") or None
    if tmpdir:
        os.makedirs(tmpdir, exist_ok=True)
    res = run_bass_kernel_spmd(nc, in_maps, list(range(N_CORES)), tmpdir=tmpdir)
    global _last_exec_ns
    _last_exec_ns = res.exec_time_ns

    out = np.empty((N_SIM, N_STEP, 1), np.float32)
    for c in range(N_CORES):
        out[c * B_CORE:(c + 1) * B_CORE, :, 0] = res.results[c]["D"].T
    return out
